# revision 49
# baseline (speedup 1.0000x reference)
r"""DetCon (NT-Xent style) contrastive loss on 8 Trainium2 NeuronCores.

Reference: v0/v1 L2-normalized (over E) scaled by 1/sqrt(T);
  logits = [[S01, S00\diag], [S10, S11\diag]]  (2BN x 2BN-1)
  loss = mean_i(logsumexp(row_i) - label_i),  label_i = S01[i,i].

Per-core plan (data-parallel rows; host np.roll makes the program
core-independent; each core's rows are cols 0..511 of each view):
  setup(r):  8x 1MB DMA raw [E, BN] f32 (sync queue); squares (DVE +
    gpsimd, bf16); per-column sumsq via ones-matmul (bf16, 1 cyc/row)
    -> [1,2048] PSUM; tiny reshape-DMA -> [8,512] SBUF; ACT Ln+Exp
    (one table set) -> scl; one-hot-row broadcast matmuls (f32r) ->
    pb [128,2048] PSUM; DVE raw*pb -> bf16 nrm.
  main(r):   per 128-row block x 2048-key tile: 8 bf16 matmuls K=256
    -> PSUM; ACT exp with accum_out row-sums (32 tiles = the ACT
    bottleneck, ~2.04us each).
  epilogue:  row-sums - exp(10) (same-view diag is exactly 10, so no
    diag extraction); ln via ACT with bias=-exp(10), accum_out;
    partition-reduce via ones-matmul; labels = colsum(nrm0*nrm1).
Reps are software-pipelined: setup(r) emission is interleaved into
main(r-1) so every engine queue stays busy; steady state is ACT-bound.
Host sums the 8 per-core partials and divides by 2*B*N.
"""

import math
from contextlib import ExitStack

import numpy as np

import concourse.bacc as bacc
import concourse.bass as bass
import concourse.tile as tile
from concourse import mybir
from concourse.bass_utils import run_bass_kernel_spmd

B, E, N = 64, 256, 64
BN = B * N            # 4096 rows per view
NCORES = 8
P = 128
KH = E // P           # 2 contraction halves
G = 2048              # column group width (setup granularity)
PW = 2048             # PSUM tile free width
NJH = G // PW         # PSUM tiles per column group
PSUM_BUFS = 16384 // (PW * 4)
NG = BN // G          # 2 column groups
GB = B // NG          # b-range per column group
TEMP = 0.1
SCALE_BIAS = -0.5 * math.log(TEMP)   # exp(-0.5*ln(s) + BIAS) = sqrt(10/s)
EXP10 = float(np.exp(np.float64(10.0)))  # exact same-view diag: |q|^2 = 10

F32 = mybir.dt.float32
F32R = mybir.dt.float32r
BF16 = mybir.dt.bfloat16
FP8 = mybir.dt.float8e4
AFT = mybir.ActivationFunctionType

# We alternate Ln and Exp on the ACT engine every rep. The table-load
# inserter picks the first set containing each function, which puts Ln and
# Exp in different sets and forces a ~1.3us ACT table reload per switch.
# Hide Exp/Ln from every set except the one that contains both, so all
# activations share one resident table (json set indices are preserved).
_orig_gat = bacc.get_activation_tables


def _gat_ln_exp_combined(arch):
    tabs = {k: set(v) for k, v in _orig_gat(arch).items()}
    for name, s in tabs.items():
        if name != "natural_log_exp_and_others":
            s.discard(AFT.Exp)
            s.discard(AFT.Ln)
    return tabs


bacc.get_activation_tables = _gat_ln_exp_combined


def _main_tile_list():
    """(g, half, m, tg) in emission order: 16 g0 tiles then 16 g1 tiles."""
    out = []
    for g in range(NG):
        for half in range(2):
            for m in range(4):
                for tg in range(2):
                    out.append((g, half, m, tg))
    return out


class _Emitter:
    def __init__(self, nc, pl):
        self.nc = nc
        self.pl = pl
        self.ones_col = None
        self.ones_col_b = None
        self.onesel = None
        # per-rep state
        self.raw = {}     # r -> [v][h] tiles
        self.sq = {}      # (r, g) -> {(v, h): tile}
        self.nrm = {}     # r -> [v][h] tiles
        self.scl = {}     # (r, g) -> scl16 tile
        self.stats = {}   # r -> stats tile
        self.lbl2 = {}    # r -> 2*sum(labels) tile

    def emit_consts(self):
        nc, pl = self.nc, self.pl
        self.ones_col = pl["cst"].tile([P, 1], F32, tag="ones_col",
                                       name="ones_col")
        nc.vector.memset(self.ones_col[:], 1.0)
        self.ones_col_b = pl["cst"].tile([P, 1], BF16, tag="ones_col_b",
                                         name="ones_col_b")
        nc.vector.memset(self.ones_col_b[:], 1.0)
        self.ones_row = pl["cst"].tile([1, P], BF16, tag="ones_row",
                                       name="ones_row")
        nc.vector.memset(self.ones_row[:], 1.0)
        self.sbias = pl["cst"].tile([8, 1], F32, tag="sbias", name="sbias")
        nc.vector.memset(self.sbias[:], SCALE_BIAS)
        self.nexp10 = pl["cst"].tile([P, 1], F32, tag="nexp10", name="nexp10")
        nc.vector.memset(self.nexp10[:], -EXP10)
        self.zbias = pl["cst"].tile([8, 1], F32, tag="zbias", name="zbias")
        nc.vector.memset(self.zbias[:], 0.0)

    # ---- setup pieces -------------------------------------------------
    def setup_dma_and_sq_g0(self, r, vin):
        """Raw loads for the whole rep + squares for g0 (and gpsimd g1)."""
        nc, pl = self.nc, self.pl
        raw = [[pl["raw"].tile([P, BN], F32, tag=f"raw{v}{h}",
                               name=f"raw{v}{h}_{r}")
                for h in range(KH)] for v in range(2)]
        self.raw[r] = raw
        for g in range(NG):
            for v in range(2):
                for h in range(KH):
                    src = vin[v][g * GB:(g + 1) * GB, h * P:(h + 1) * P, :] \
                        .rearrange("b e n -> e b n")
                    dst = raw[v][h][:, g * G:(g + 1) * G].rearrange(
                        "e (b n) -> e b n", b=GB)
                    nc.sync.dma_start(out=dst, in_=src)
        # fp8 DoubleRow layout: [K=128, k-subtile, col] per view
        self.nrm[r] = [pl["nrm"].tile([P, KH, BN], FP8, tag=f"nrm{v}",
                                      name=f"nrm{v}_{r}")
                       for v in range(2)]
        # squares: h==0 on DVE, h==1 on gpsimd; gpsimd also takes g1 now
        self._emit_sq(r, 0, engines=("vector", "gpsimd"))
        self._emit_sq(r, 1, engines=(None, "gpsimd"))

    def _emit_sq(self, r, g, engines):
        nc, pl = self.nc, self.pl
        d = self.sq.setdefault((r, g), {})
        gs = slice(g * G, (g + 1) * G)
        for v in range(2):
            for h in range(KH):
                eng = engines[h]
                if eng is None or (v, h) in d:
                    continue
                t = pl["sq"].tile([P, G], BF16, tag="sq", name=f"sq{v}{h}{g}_{r}")
                getattr(nc, eng).tensor_mul(
                    t[:], self.raw[r][v][h][:, gs], self.raw[r][v][h][:, gs])
                d[(v, h)] = t

    def setup_colsum_g(self, r, g):
        """Per-column sumsq matmuls -> PSUM row -> SBUF [8,512] stage."""
        nc, pl = self.nc, self.pl
        if g == 0:
            self._emit_sq(r, 1, engines=("vector", None))
        sq = self.sq[(r, g)]
        sstg = pl["stg"].tile([8, 512], F32, tag="sstg", name=f"sstg{g}_{r}")
        for v in range(2):
            sres = pl["stg"].tile([1, G], F32, tag="sres",
                                  name=f"sres{v}{g}_{r}")
            for bh in range(NJH):
                ss = pl["psum"].tile([P, PW], F32, tag="ps",
                                     name=f"ss{v}{g}{bh}_{r}")
                for b in range(PW // 512):
                    js = slice(b * 512, (b + 1) * 512)
                    for h in range(KH):
                        nc.tensor.matmul(
                            ss[0:1, js], self.ones_col_b[:],
                            sq[(v, h)][:, bh * PW + b * 512:
                                       bh * PW + (b + 1) * 512],
                            start=(h == 0), stop=(h == KH - 1))
                # PSUM -> SBUF bounce (DMA can't read PSUM)
                nc.vector.tensor_copy(
                    sres[0:1, bh * PW:(bh + 1) * PW], ss[0:1, :])
            # reshape [1,2048] -> [4,512] rows so Ln/Exp use 8 ACT lanes
            nc.gpsimd.dma_start(out=sstg[v * 4:(v + 1) * 4, :], in_=sres[:])
        self.scl[(r, g, "sstg")] = sstg

    def setup_scale_g(self, r, g):
        """Ln/Exp -> stride-0 broadcast DMA into SBUF pb -> apply."""
        nc, pl = self.nc, self.pl
        sstg = self.scl.pop((r, g, "sstg"))
        gs = slice(g * G, (g + 1) * G)
        lnstg = pl["stg"].tile([8, 512], F32, tag="lnstg", name=f"ln{g}_{r}")
        nc.scalar.activation(lnstg[:], sstg[:], AFT.Ln, bias=self.zbias[:])
        scl16 = pl["stg"].tile([8, 512], BF16, tag="scl16", name=f"scl{g}_{r}")
        nc.scalar.activation(scl16[:], lnstg[:], AFT.Exp,
                             scale=-0.5, bias=self.sbias[:])
        self.scl[(r, g)] = scl16
        # broadcast each scale row across 128 partitions via DMA (keeps the
        # PE free of any dependency on the ACT-produced scales)
        for v in range(2):
            srow = pl["stg"].tile([1, G], BF16, tag="srow",
                                  name=f"srow{v}{g}_{r}")
            nc.gpsimd.dma_start(out=srow[:], in_=scl16[v * 4:(v + 1) * 4, :])
            pb = pl["pbs"].tile([P, G], BF16, tag=f"pb{v}",
                                name=f"pb{v}{g}_{r}")
            nc.gpsimd.partition_broadcast(pb[:], srow[0:1, :])
            for h in range(KH):
                nc.vector.tensor_mul(
                    self.nrm[r][v][:, h, gs], self.raw[r][v][h][:, gs], pb[:])

    def emit_label(self, r):
        """2 * sum_i(label_i) for this core's 512 rows (bf16 path)."""
        nc, pl = self.nc, self.pl
        nrm = self.nrm[r]
        tmps = []
        for h in range(KH):
            t = pl["sml"].tile([P, 512], BF16, tag=f"lblt{h}",
                               name=f"lblt{h}_{r}")
            nc.vector.tensor_mul(t[:], nrm[0][:, h, 0:512],
                                 nrm[1][:, h, 0:512])
            tmps.append(t)
        lbl = pl["psum"].tile([P, PW], F32, tag="ps", name=f"lbl_{r}")
        for h in range(KH):
            nc.tensor.matmul(lbl[0:1, 0:512], self.ones_col_b[:], tmps[h][:],
                             start=(h == 0), stop=(h == KH - 1))
        lbls = pl["sml"].tile([1, 1], F32, tag="lbls", name=f"lbls_{r}")
        nc.vector.tensor_reduce(lbls[:], lbl[0:1, 0:512],
                                axis=mybir.AxisListType.X,
                                op=mybir.AluOpType.add)
        lbl2 = pl["sml"].tile([1, 1], F32, tag="lbl2", name=f"lbl2_{r}")
        nc.vector.tensor_scalar_mul(lbl2[:], lbls[:], 2.0)
        self.lbl2[r] = lbl2

    # ---- main pieces --------------------------------------------------
    def main_tiles(self, r, tiles):
        nc, pl = self.nc, self.pl
        nrm = self.nrm[r]
        if r not in self.stats:
            self.stats[r] = pl["sml"].tile([P, 32 * NJH], F32, tag="stats",
                                           name=f"stats_{r}")
        stats = self.stats[r]
        for (g, half, m, tg) in tiles:
            ms = slice(m * P, (m + 1) * P)
            keys = nrm[1 - half] if tg == 0 else nrm[half]
            for jh in range(NJH):
                goff = g * G + jh * PW
                pt = pl["psum"].tile([P, PW], F32, tag="ps",
                                     name=f"pt{g}{half}{m}{tg}{jh}_{r}")
                for j in range(PW // 512):
                    js = slice(j * 512, (j + 1) * 512)
                    nc.tensor.matmul(
                        pt[:, js], nrm[half][:, :, ms],
                        keys[:, :, goff + j * 512: goff + (j + 1) * 512],
                        perf_mode=mybir.MatmulPerfMode.DoubleRow)
                esc = pl["esc"].tile([P, PW], BF16, tag="esc",
                                     name=f"esc{g}{half}{m}{tg}{jh}_{r}")
                sidx = (((half * 4 + m) * 2 + tg) * 2 + g) * NJH + jh
                nc.scalar.activation(esc[:], pt[:, :], AFT.Exp,
                                     accum_out=stats[:, sidx:sidx + 1])

    def emit_epilogue(self, r, out_dram):
        nc, pl = self.nc, self.pl
        stats = self.stats[r]
        rows = pl["sml"].tile([P, 8], F32, tag="rows", name=f"rows_{r}")
        nc.vector.tensor_reduce(
            rows[:], stats[:].rearrange("p (m t) -> p m t", t=4 * NJH),
            axis=mybir.AxisListType.X, op=mybir.AluOpType.add)
        lnr = pl["sml"].tile([P, 8], F32, tag="lnr", name=f"lnr_{r}")
        lnsum = pl["sml"].tile([P, 1], F32, tag="lnsum", name=f"lnsum_{r}")
        # ln(rowsum - exp(10)): removes the same-view diag term exactly
        nc.scalar.activation(lnr[:], rows[:], AFT.Ln, bias=self.nexp10[:],
                             accum_out=lnsum[:])
        fp = pl["psum"].tile([P, PW], F32, tag="ps", name=f"fp_{r}")
        nc.tensor.matmul(fp[0:1, 0:1], lnsum[:], self.ones_col[:])
        res = pl["sml"].tile([1, 1], F32, tag="res", name=f"res_{r}")
        nc.vector.tensor_sub(res[:], fp[0:1, 0:1], self.lbl2[r][:])
        nc.gpsimd.dma_start(out=out_dram[:], in_=res[:])
        # free per-rep references
        for d in (self.raw, self.nrm, self.stats, self.lbl2):
            d.pop(r, None)
        for g in range(NG):
            self.sq.pop((r, g), None)
            self.scl.pop((r, g), None)


def _build_nc(reps: int = 1, mode: str = "full"):
    nc = bacc.Bacc()
    vin = [
        nc.dram_tensor("view0", [B, E, N], F32, kind="ExternalInput"),
        nc.dram_tensor("view1", [B, E, N], F32, kind="ExternalInput"),
    ]
    out_dram = nc.dram_tensor("out", [1, 1], F32, kind="ExternalOutput")

    with ExitStack() as ctx:
        tc = ctx.enter_context(tile.TileContext(nc))
        pl = {
            "raw": ctx.enter_context(tc.tile_pool(name="raw", bufs=1)),
            "nrm": ctx.enter_context(tc.tile_pool(name="nrm", bufs=2)),
            "sq": ctx.enter_context(tc.tile_pool(name="sq", bufs=5)),
            "esc": ctx.enter_context(tc.tile_pool(name="esc", bufs=1)),
            "pbs": ctx.enter_context(tc.tile_pool(name="pbs", bufs=2)),
            "stg": ctx.enter_context(tc.tile_pool(name="stg", bufs=2)),
            "sml": ctx.enter_context(tc.tile_pool(name="sml", bufs=2)),
            "cst": ctx.enter_context(tc.tile_pool(name="cst", bufs=1)),
            "psum": ctx.enter_context(
                tc.tile_pool(name="psum", bufs=PSUM_BUFS, space="PSUM")),
        }
        em = _Emitter(nc, pl)
        em.emit_consts()
        tiles = _main_tile_list()
        if mode == "main":
            # one setup, then reps x main phase (timing attribution)
            em.setup_dma_and_sq_g0(0, vin)
            for g in range(NG):
                em.setup_colsum_g(0, g)
                em.setup_scale_g(0, g)
            em.emit_label(0)
            nrm0, lbl20 = em.nrm[0], em.lbl2[0]
            for r in range(reps):
                em.nrm[r], em.lbl2[r] = nrm0, lbl20
                em.main_tiles(r, tiles)
                em.emit_epilogue(r, out_dram)
                em.nrm[0], em.lbl2[0] = nrm0, lbl20
            nc.compile()
            return nc
        if mode == "setup":
            for r in range(reps):
                em.setup_dma_and_sq_g0(r, vin)
                for g in range(NG):
                    em.setup_colsum_g(r, g)
                    em.setup_scale_g(r, g)
                em.emit_label(r)
                del em.raw[r], em.nrm[r]
            nc.compile()
            return nc
        for r in range(reps):
            em.setup_dma_and_sq_g0(r, vin)
            if r > 0:
                em.main_tiles(r - 1, tiles[0:4])
            if r > 1:
                # deferred epilogue: keeps the r-2 -> r-1 ACT boundary
                # free of the rows/lnr/fp serial chain
                em.emit_epilogue(r - 2, out_dram)
            if r > 0:
                em.main_tiles(r - 1, tiles[4:12])
            em.setup_colsum_g(r, 0)
            if r > 0:
                em.main_tiles(r - 1, tiles[12:16])
            em.setup_scale_g(r, 0)
            if r > 0:
                em.main_tiles(r - 1, tiles[16:18])
            em.setup_colsum_g(r, 1)
            if r > 0:
                em.main_tiles(r - 1, tiles[18:24])
            if r > 0:
                em.main_tiles(r - 1, tiles[24:26])
            em.setup_scale_g(r, 1)
            if r > 0:
                em.main_tiles(r - 1, tiles[26:32])
            em.emit_label(r)
        if reps > 1:
            em.emit_epilogue(reps - 2, out_dram)
        em.main_tiles(reps - 1, tiles)
        em.emit_epilogue(reps - 1, out_dram)

    nc.compile()
    return nc


_NC_CACHE = None


def _run_spmd(view0: np.ndarray, view1: np.ndarray, nc=None, **spmd_kwargs):
    global _NC_CACHE
    if nc is None:
        if _NC_CACHE is None:
            _NC_CACHE = _build_nc()
        nc = _NC_CACHE

    in_maps = []
    for c in range(NCORES):
        in_maps.append({
            "view0": np.ascontiguousarray(
                np.roll(view0, -c * (B // NCORES), axis=0)),
            "view1": np.ascontiguousarray(
                np.roll(view1, -c * (B // NCORES), axis=0)),
        })
    res = run_bass_kernel_spmd(nc, in_maps, core_ids=list(range(NCORES)),
                               **spmd_kwargs)
    total = sum(float(r["out"][0, 0]) for r in res.results)
    return np.float32(total / (2 * BN)), res


def kernel(view0: np.ndarray, view1: np.ndarray) -> np.ndarray:
    loss, _ = _run_spmd(view0, view1)
    return loss
